# revision 20
# baseline (speedup 1.0000x reference)
"""Matryoshka transformer Bass kernel for TRN2.

Sharding: data-parallel over batch B=8 -> 8 cores, 1 sequence per core.
Layout: activations feature-major (xT: [dims on partitions, tokens on free]).
All compute fp32. Weights pre-tiled on host for contiguous DMA.
"""
import sys
sys.path.insert(0, "/opt/trn_rl_repo")
sys.path.insert(0, "/opt/trn_rl_repo/concourse")
import numpy as np

import concourse.bass as bass
import concourse.bacc as bacc
import concourse.mybir as mybir
import concourse.tile as tile
from concourse.bass_utils import run_bass_kernel_spmd

F32 = mybir.dt.float32
AF = mybir.ActivationFunctionType

T = 1024            # tokens per core (one batch element)
D = 1024            # model dim
TIER = [256, 768]
NH = 16             # total heads
DK = 64
FF = 4096           # total ff dim (1024 + 3072)
NL = 2
QT = 512            # token tile (free dim of most matmuls)
EPS = 1e-5
VSTR = 68           # per-head stride in V buffer (64 dims + 1 ones + 3 pad)

_CACHE = {}
DBG = False
DBG_STOP = 0


def _tier_kts(j):
    """Contraction k-tiles (of the 8 input d-tiles) for output d-tile j of a
    block-upper-triangular [1024 -> 1024] matmul (tier boundary at 256 = 2 tiles)."""
    return [0, 1] if j < 2 else list(range(8))


def _fc1_kts(f):
    return [0, 1] if f < 8 else list(range(8))


def _fc2_kts(j, half):
    if j < 2:
        return list(range(8)) if half == 0 else []
    return list(range(half * 16, half * 16 + 16))


class _StopBuild(Exception):
    pass


def build_nc():
    nc = bacc.Bacc("TRN2", target_bir_lowering=False, debug=False)

    # ---- DRAM I/O ----
    x_d = nc.dram_tensor("x", (8, 128, T), F32, kind="ExternalInput")
    out_d = nc.dram_tensor("out", (8, 128, T), F32, kind="ExternalOutput")
    utmask_d = nc.dram_tensor("utmask", (128, 128), F32, kind="ExternalInput")
    selmat_d = nc.dram_tensor("selmat", (2, 512), F32, kind="ExternalInput")
    onesab_d = nc.dram_tensor("onesab", (128, 4), F32, kind="ExternalInput")
    lnsc_d = nc.dram_tensor("lnsc", (2, 3), F32, kind="ExternalInput")
    dbg = {}
    if DBG:
        dbg["h"] = nc.dram_tensor("dbg_h", (8, 128, 512), F32, kind="ExternalOutput")
        dbg["q"] = nc.dram_tensor("dbg_q", (8, 128, 512), F32, kind="ExternalOutput")
        dbg["k"] = nc.dram_tensor("dbg_k", (8, 128, 1024), F32, kind="ExternalOutput")
        dbg["v"] = nc.dram_tensor("dbg_v", (8, 128, 16 * VSTR), F32, kind="ExternalOutput")
        dbg["o"] = nc.dram_tensor("dbg_o", (8, 128, 512), F32, kind="ExternalOutput")
        dbg["x1"] = nc.dram_tensor("dbg_x1", (8, 128, 1024), F32, kind="ExternalOutput")
        dbg["c"] = nc.dram_tensor("dbg_c", (64, 512), F32, kind="ExternalOutput")
        dbg["f"] = nc.dram_tensor("dbg_f", (16, 128, 512), F32, kind="ExternalOutput")
        dbg["x0"] = nc.dram_tensor("dbg_x0", (8, 128, 1024), F32, kind="ExternalOutput")
        dbg["x2"] = nc.dram_tensor("dbg_x2", (8, 128, 1024), F32, kind="ExternalOutput")
    W = []
    for l in range(NL):
        W.append({
            "wq": nc.dram_tensor(f"wq{l}", (8, 128, 1024), F32, kind="ExternalInput"),
            "wk": nc.dram_tensor(f"wk{l}", (8, 128, 1024), F32, kind="ExternalInput"),
            "wv": nc.dram_tensor(f"wv{l}", (8, 128, 1024), F32, kind="ExternalInput"),
            "wo": nc.dram_tensor(f"wo{l}", (8, 128, 1024), F32, kind="ExternalInput"),
            "w1": nc.dram_tensor(f"w1{l}", (32, 128, 1024), F32, kind="ExternalInput"),
            "w2": nc.dram_tensor(f"w2{l}", (8, 128, 4096), F32, kind="ExternalInput"),
            "fkv": nc.dram_tensor(f"fkv{l}", (128, 384), F32, kind="ExternalInput"),
            "pkv": nc.dram_tensor(f"pkv{l}", (64, 512), F32, kind="ExternalInput"),
        })

    with tile.TileContext(nc) as tc:
        with (
            tc.tile_pool(name="xp", bufs=8) as xp,
            tc.tile_pool(name="big", bufs=16) as bigp,
            tc.tile_pool(name="hp", bufs=8) as hp,
            tc.tile_pool(name="qp", bufs=8) as qp,
            tc.tile_pool(name="op", bufs=8) as op_,
            tc.tile_pool(name="wp", bufs=3) as wp,
            tc.tile_pool(name="pp", bufs=5) as pp,
            tc.tile_pool(name="smp", bufs=4) as smp,
            tc.tile_pool(name="cst", bufs=1) as cst,
            tc.tile_pool(name="ps", bufs=4, space="PSUM") as psp,
            tc.tile_pool(name="pstat", bufs=2, space="PSUM") as pstat,
            tc.tile_pool(name="pso", bufs=2, space="PSUM") as pso,
        ):
            # ---- constants ----
            ut_t = cst.tile([128, 128], F32, tag="ut")
            nc.sync.dma_start(ut_t[:], utmask_d[:])
            sel_t = cst.tile([2, 512], F32, tag="sel")
            nc.sync.dma_start(sel_t[:], selmat_d[:])
            oab_t = cst.tile([128, 4], F32, tag="oab")
            nc.sync.dma_start(oab_t[:], onesab_d[:])
            lnsc_t = cst.tile([2, 3], F32, tag="lnsc")
            nc.sync.dma_start(lnsc_t[:], lnsc_d[:])
            ones64_t = cst.tile([128, 64], F32, tag="o64")
            nc.gpsimd.memset(ones64_t[:], 1.0)

            # ---- persistent x tiles ----
            x_t = []
            for kt in range(8):
                xt = xp.tile([128, T], F32, tag="x")
                nc.sync.dma_start(xt[:], x_d[kt])
                x_t.append(xt)

            if DBG:
                for i in range(8):
                    nc.sync.dma_start(dbg["x0"][i], x_t[i][:])
            try:
              for l in range(NL):
                w = W[l]
                fkv_t = cst.tile([128, 6 * 64], F32, tag="fkv", name="fkv_t", bufs=2)
                nc.sync.dma_start(fkv_t[:], w["fkv"][:])
                pkv_t = cst.tile([64, 512], F32, tag="pkv", name="pkv_t", bufs=2)
                nc.sync.dma_start(pkv_t[:], w["pkv"][:])

                # ========== LN1 + QKV + attention + wo, per q-tile ==========
                k_t = [bigp.tile([128, T], F32, tag="big", name=f"k{i}") for i in range(8)]
                v_t = [bigp.tile([128, 16 * VSTR], F32, tag="big", name=f"v{i}") for i in range(8)]
                for tb in range(8):
                    nc.gpsimd.memset(v_t[tb][:], 0.0)
                    nc.gpsimd.memset(
                        v_t[tb][:].rearrange("p (h c) -> p h c", c=VSTR)[:, :, 64:65],
                        1.0,
                    )

                def ln_block(qt, dst_h):
                    """dst_h: list of 8 [128,512] tiles = per-tier LN of x[:, qt]."""
                    cols = slice(qt * QT, (qt + 1) * QT)
                    st_ps = pstat.tile([2, QT], F32, tag="stat")
                    sq_ps = pstat.tile([2, QT], F32, tag="stat")
                    for kt in range(8):
                        lo = 0 if kt < 2 else 2
                        lhs1 = oab_t[:, lo:lo + 2]
                        nc.tensor.matmul(
                            st_ps[:], lhs1, x_t[kt][:, cols],
                            start=(kt == 0), stop=(kt == 7),
                        )
                        sq = pp.tile([128, QT], F32, tag="p")
                        nc.vector.tensor_mul(sq[:], x_t[kt][:, cols], x_t[kt][:, cols])
                        nc.tensor.matmul(
                            sq_ps[:], lhs1, sq[:],
                            start=(kt == 0), stop=(kt == 7),
                        )
                    st_sb = smp.tile([2, QT], F32, tag="st")
                    sq_sb = smp.tile([2, QT], F32, tag="st")
                    nc.vector.tensor_copy(st_sb[:], st_ps[:])
                    nc.vector.tensor_copy(sq_sb[:], sq_ps[:])
                    s2d = smp.tile([2, QT], F32, tag="st")
                    nc.scalar.activation(s2d[:], st_sb[:], AF.Square,
                                         scale=lnsc_t[:, 0:1])
                    u = smp.tile([2, QT], F32, tag="st")
                    nc.vector.tensor_sub(u[:], sq_sb[:], s2d[:])
                    sv = smp.tile([2, QT], F32, tag="st")
                    nc.scalar.activation(sv[:], u[:], AF.Sqrt,
                                         scale=lnsc_t[:, 1:2],
                                         bias=lnsc_t[:, 2:3])
                    rstd = smp.tile([2, QT], F32, tag="st")
                    nc.vector.reciprocal(rstd[:], sv[:])
                    pr = smp.tile([2, QT], F32, tag="st")
                    nc.vector.tensor_mul(pr[:], st_sb[:], rstd[:])
                    for tier in (0, 1):
                        rb = psp.tile([128, QT], F32, tag="ps")
                        nc.tensor.matmul(rb[:], sel_t[:, tier * 128:(tier + 1) * 128],
                                         rstd[:], start=True, stop=True)
                        mb = psp.tile([128, QT], F32, tag="ps")
                        nc.tensor.matmul(mb[:], sel_t[:, (2 + tier) * 128:(3 + tier) * 128],
                                         pr[:], start=True, stop=True)
                        for kt in (range(0, 2) if tier == 0 else range(2, 8)):
                            nc.vector.tensor_mul(dst_h[kt][:], x_t[kt][:, cols], rb[:])
                            nc.vector.tensor_add(dst_h[kt][:], dst_h[kt][:], mb[:])

                for qt in range(2):
                    cols = slice(qt * QT, (qt + 1) * QT)
                    h_t = [hp.tile([128, QT], F32, tag="h", name=f"h{i}") for i in range(8)]
                    ln_block(qt, h_t)

                    if DBG and l == 0 and qt == 0:
                        for i in range(8):
                            nc.sync.dma_start(dbg["h"][i], h_t[i][:])
                    if DBG_STOP == 1:
                        raise _StopBuild
                    # ---- low-rank feedback coefficients c_k (rows 0-31), c_v (32-63)
                    c_ps = psp.tile([64, QT], F32, tag="ps")
                    for a in range(6):
                        nc.tensor.matmul(
                            c_ps[:], fkv_t[:, a * 64:(a + 1) * 64], h_t[2 + a][:],
                            start=(a == 0), stop=(a == 5),
                        )
                    c_sb = smp.tile([64, QT], F32, tag="c", bufs=2)
                    nc.vector.tensor_copy(c_sb[:], c_ps[:])

                    # ---- K (feature-major) ----
                    for j in range(8):
                        wt = wp.tile([128, 1024], F32, tag="w")
                        nc.sync.dma_start(wt[:], w["wk"][j])
                        kts = _tier_kts(j)
                        has_fb = j < 2
                        kps = psp.tile([128, QT], F32, tag="ps")
                        for i, a in enumerate(kts):
                            nc.tensor.matmul(
                                kps[:], wt[:, a * 128:(a + 1) * 128], h_t[a][:],
                                start=(i == 0),
                                stop=(i == len(kts) - 1 and not has_fb),
                            )
                        if has_fb:
                            nc.tensor.matmul(
                                kps[:], pkv_t[0:32, j * 128:(j + 1) * 128],
                                c_sb[0:32, :], start=False, stop=True,
                            )
                        nc.vector.tensor_copy(k_t[j][:, cols], kps[:])

                    # ---- V (token-major, strided with ones col) ----
                    vchunks = [(0, 256, [0, 1]), (256, 768, list(range(8))),
                               (768, 1024, list(range(8)))]
                    for (c0, c1, kts) in vchunks:
                        n = c1 - c0
                        vps = [psp.tile([128, 512], F32, tag="ps", name=f"vps{i}") for i in range(4)]
                        for i, a in enumerate(kts):
                            wt = wp.tile([128, 1024], F32, tag="w")
                            nc.sync.dma_start(wt[:, 0:n], w["wv"][a][:, c0:c1])
                            for tb in range(4):
                                nc.tensor.matmul(
                                    vps[tb][:, 0:n], h_t[a][:, tb * 128:(tb + 1) * 128],
                                    wt[:, 0:n],
                                    start=(i == 0), stop=(i == len(kts) - 1 and c0 != 0),
                                )
                        if c0 == 0:  # feedback adds into dims 0-255
                            for tb in range(4):
                                nc.tensor.matmul(
                                    vps[tb][:, 0:n],
                                    c_sb[32:64, tb * 128:(tb + 1) * 128],
                                    pkv_t[32:64, 256:512],
                                    start=False, stop=True,
                                )
                        for tb in range(4):
                            gtb = qt * 4 + tb
                            hv = v_t[gtb][:].rearrange("p (h c) -> p h c", c=VSTR)
                            h0, h1 = c0 // 64, c1 // 64
                            dst = hv[:, h0:h1, 0:64]
                            src = vps[tb][:, 0:n].rearrange("p (h c) -> p h c", c=64)
                            nc.vector.tensor_copy(dst, src)

                    # ---- Q (feature-major; 0.125 scale folded into wq) ----
                    q_t = [qp.tile([128, QT], F32, tag="q", name=f"q{i}") for i in range(8)]
                    for j in range(8):
                        wt = wp.tile([128, 1024], F32, tag="w")
                        nc.sync.dma_start(wt[:], w["wq"][j])
                        kts = _tier_kts(j)
                        qps = psp.tile([128, QT], F32, tag="ps")
                        for i, a in enumerate(kts):
                            nc.tensor.matmul(
                                qps[:], wt[:, a * 128:(a + 1) * 128], h_t[a][:],
                                start=(i == 0), stop=(i == len(kts) - 1),
                            )
                        nc.vector.tensor_copy(q_t[j][:], qps[:])

                    if DBG and l == 0 and qt == 0:
                        nc.sync.dma_start(dbg["c"][:], c_sb[:])
                        for i in range(8):
                            nc.sync.dma_start(dbg["q"][i], q_t[i][:])
                    if DBG_STOP == 2:
                        raise _StopBuild
                    # ---- attention for this q-tile ----
                    o_t = [op_.tile([128, QT], F32, tag="o", name=f"o{i}") for i in range(8)]
                    nkb = 4 * (qt + 1)
                    for hd in range(16):
                        j, r0 = hd // 2, (hd % 2) * 64
                        p_tiles = []
                        for kb in range(nkb):
                            sps = psp.tile([128, QT], F32, tag="ps")
                            nc.tensor.matmul(
                                sps[:],
                                k_t[j][r0:r0 + 64, kb * 128:(kb + 1) * 128],
                                q_t[j][r0:r0 + 64, :],
                                start=True, stop=True,
                            )
                            pt = pp.tile([128, QT], F32, tag="p")
                            c0 = kb * 128 - qt * QT
                            if c0 < 0:
                                nc.scalar.activation(pt[:], sps[:], AF.Exp)
                            else:
                                if c0 > 0:
                                    nc.gpsimd.memset(pt[:, 0:c0], 0.0)
                                nc.scalar.activation(pt[:, c0:QT], sps[:, c0:QT], AF.Exp)
                                nc.vector.tensor_mul(
                                    pt[:, c0:c0 + 128], pt[:, c0:c0 + 128], ut_t[:]
                                )
                            p_tiles.append(pt)
                        ops = pso.tile([68, QT], F32, tag="po")
                        for kb in range(nkb):
                            nc.tensor.matmul(
                                ops[:],
                                v_t[kb][:, hd * VSTR:hd * VSTR + 68],
                                p_tiles[kb][:],
                                start=(kb == 0), stop=(kb == nkb - 1),
                            )
                        o65 = pp.tile([128, QT], F32, tag="p")
                        nc.scalar.activation(o65[0:65, :], ops[0:65, :], AF.Copy)
                        dnb = psp.tile([64, QT], F32, tag="ps")
                        nc.tensor.matmul(dnb[:], ones64_t[64:65, :], o65[64:65, :],
                                         start=True, stop=True)
                        rcp = pp.tile([128, QT], F32, tag="p")
                        nc.vector.reciprocal(rcp[0:64, :], dnb[:])
                        nc.vector.tensor_mul(
                            o_t[j][r0:r0 + 64, :], o65[0:64, :], rcp[0:64, :]
                        )

                    # ---- wo + residual ----
                    if DBG and l == 0 and qt == 0:
                        for i in range(8):
                            nc.sync.dma_start(dbg["o"][i], o_t[i][:])
                    if DBG_STOP == 3:
                        raise _StopBuild
                    for j in range(8):
                        wt = wp.tile([128, 1024], F32, tag="w")
                        nc.sync.dma_start(wt[:], w["wo"][j])
                        kts = _tier_kts(j)
                        aps = psp.tile([128, QT], F32, tag="ps")
                        for i, a in enumerate(kts):
                            nc.tensor.matmul(
                                aps[:], wt[:, a * 128:(a + 1) * 128], o_t[a][:],
                                start=(i == 0), stop=(i == len(kts) - 1),
                            )
                        nc.vector.tensor_add(x_t[j][:, cols], x_t[j][:, cols], aps[:])

                if DBG and l == 0:
                    for i in range(8):
                        nc.sync.dma_start(dbg["k"][i], k_t[i][:])
                        nc.sync.dma_start(dbg["v"][i], v_t[i][:])
                        nc.sync.dma_start(dbg["x1"][i], x_t[i][:])
                if DBG_STOP == 4:
                    raise _StopBuild
                # ========== LN2 + MLP, per q-tile ==========
                for qt in range(2):
                    cols = slice(qt * QT, (qt + 1) * QT)
                    h_t = [hp.tile([128, QT], F32, tag="h", name=f"h{i}") for i in range(8)]
                    ln_block(qt, h_t)
                    part = [None] * 8
                    for half in range(2):
                        f_sb = []
                        for fl in range(16):
                            f = half * 16 + fl
                            wt = wp.tile([128, 1024], F32, tag="w")
                            nc.sync.dma_start(wt[:], w["w1"][f])
                            kts = _fc1_kts(f)
                            fps = psp.tile([128, QT], F32, tag="ps")
                            for i, a in enumerate(kts):
                                nc.tensor.matmul(
                                    fps[:], wt[:, a * 128:(a + 1) * 128], h_t[a][:],
                                    start=(i == 0), stop=(i == len(kts) - 1),
                                )
                            ft = bigp.tile([128, QT], F32, tag="big")
                            nc.scalar.activation(ft[:], fps[:], AF.Gelu)
                            if DBG and l == 0 and qt == 0 and half == 0:
                                nc.sync.dma_start(dbg["f"][fl], ft[:])
                            f_sb.append(ft)
                        for j in range(8):
                            kts = _fc2_kts(j, half)
                            if not kts:
                                continue
                            nw = len(kts) // 8
                            wts = []
                            for wi in range(nw):
                                wt = wp.tile([128, 1024], F32, tag="w")
                                nc.sync.dma_start(
                                    wt[:],
                                    w["w2"][j][:, (kts[0] + wi * 8) * 128:
                                               (kts[0] + wi * 8 + 8) * 128],
                                )
                                wts.append(wt)
                            ps2 = psp.tile([128, QT], F32, tag="ps")
                            for i, a in enumerate(kts):
                                wt = wts[i // 8]
                                cc = (i % 8) * 128
                                nc.tensor.matmul(
                                    ps2[:], wt[:, cc:cc + 128], f_sb[a - half * 16][:],
                                    start=(i == 0), stop=(i == len(kts) - 1),
                                )
                            if part[j] is None:
                                part[j] = op_.tile([128, QT], F32, tag="o", name=f"part{j}")
                                nc.vector.tensor_copy(part[j][:], ps2[:])
                            else:
                                nc.vector.tensor_add(part[j][:], part[j][:], ps2[:])
                    for j in range(8):
                        nc.vector.tensor_add(x_t[j][:, cols], x_t[j][:, cols],
                                             part[j][:])
                if DBG and l == 0:
                    for i in range(8):
                        nc.sync.dma_start(dbg["x2"][i], x_t[i][:])
                if DBG_STOP == 5:
                    raise _StopBuild

            except _StopBuild:
                pass
            for kt in range(8):
                nc.sync.dma_start(out_d[kt], x_t[kt][:])

    nc.compile()
    return nc


def _col_tile(wm, nin_t, nout):
    """[nin, nout] -> [nout/128, 128, nin_t*128] pre-tiled for lhsT column use."""
    nin = nin_t * 128
    return np.ascontiguousarray(
        wm.reshape(nin_t, 128, nout // 128, 128).transpose(2, 1, 0, 3)
        .reshape(nout // 128, 128, nin)
    )


def _dense_but(blocks, ins, outs):
    m = np.zeros((sum(ins), sum(outs)), np.float32)
    io = np.concatenate([[0], np.cumsum(ins)])
    oo = np.concatenate([[0], np.cumsum(outs)])
    for i in range(len(ins)):
        for j in range(i, len(outs)):
            m[io[i]:io[i + 1], oo[j]:oo[j + 1]] = np.asarray(blocks[f"{i}_{j}"])
    return m


def _prep_weights(params):
    ins = {}
    for l, p in enumerate(params):
        g1 = np.concatenate([np.asarray(g) for g in p["ln1_g"]])
        b1 = np.concatenate([np.asarray(b) for b in p["ln1_b"]])
        g2 = np.concatenate([np.asarray(g) for g in p["ln2_g"]])
        b2 = np.concatenate([np.asarray(b) for b in p["ln2_b"]])
        assert np.abs(b1).max() == 0 and np.abs(b2).max() == 0, "LN bias fold not implemented"
        wq = _dense_but(p["wq"], TIER, TIER) * g1[:, None] * 0.125
        wk = _dense_but(p["wk"], TIER, TIER) * g1[:, None]
        wv = _dense_but(p["wv"], TIER, TIER) * g1[:, None]
        wo = _dense_but(p["wo"], TIER, TIER)
        w1 = _dense_but(p["fc1"], TIER, [1024, 3072]) * g2[:, None]
        w2 = _dense_but(p["fc2"], [1024, 3072], TIER)
        fk = np.asarray(p["fk"]) * g1[256:, None]
        fv = np.asarray(p["fv"]) * g1[256:, None]
        fkv = np.concatenate([fk, fv], 1)  # [768, 64]
        pk, pv = np.asarray(p["pk"]), np.asarray(p["pv"])
        pkv = np.zeros((64, 512), np.float32)
        for h in range(4):
            pkv[h * 8:(h + 1) * 8, h * 64:(h + 1) * 64] = pk[h]
            pkv[32 + h * 8:32 + (h + 1) * 8, 256 + h * 64:256 + (h + 1) * 64] = pv[h]
        ins[f"wq{l}"] = _col_tile(wq, 8, 1024)
        ins[f"wk{l}"] = _col_tile(wk, 8, 1024)
        ins[f"wv{l}"] = np.ascontiguousarray(wv.reshape(8, 128, 1024))
        ins[f"wo{l}"] = _col_tile(wo, 8, 1024)
        ins[f"w1{l}"] = _col_tile(w1, 8, 4096)
        ins[f"w2{l}"] = _col_tile(w2, 32, 1024)
        ins[f"fkv{l}"] = np.ascontiguousarray(
            fkv.reshape(6, 128, 64).transpose(1, 0, 2).reshape(128, 384))
        ins[f"pkv{l}"] = pkv

    ins["utmask"] = np.triu(np.ones((128, 128), np.float32))
    sel = np.zeros((2, 512), np.float32)
    sel[0, 0:128] = 1.0          # rstd bcast tier0
    sel[1, 128:256] = 1.0        # rstd bcast tier1
    sel[0, 256:384] = -1.0 / 256  # ms bcast tier0
    sel[1, 384:512] = -1.0 / 768  # ms bcast tier1
    ins["selmat"] = sel
    oab = np.zeros((128, 4), np.float32)
    oab[:, 0] = 1.0  # tier0 sum selector (lhsT cols 0:2 -> rows [1,0])
    oab[:, 3] = 1.0  # tier1 sum selector (cols 2:4 -> rows [0,1])
    ins["onesab"] = oab
    lnsc = np.array([[1 / 16.0, 1 / 256.0, EPS],
                     [1 / np.sqrt(768.0), 1 / 768.0, EPS]], np.float32)
    ins["lnsc"] = lnsc
    return ins


def kernel(x, params):
    x = np.asarray(x, np.float32)
    if "nc" not in _CACHE:
        _CACHE["nc"] = build_nc()
    nc = _CACHE["nc"]
    common = _prep_weights(params)
    in_maps = []
    for i in range(8):
        xt = np.ascontiguousarray(x[i].T.reshape(8, 128, T))
        in_maps.append({**common, "x": xt})
    res = run_bass_kernel_spmd(nc, in_maps, core_ids=list(range(8)))
    outs = []
    for i in range(8):
        o = res.results[i]["out"].reshape(1024, T).T  # [tokens, dims]
        outs.append(o)
    return np.ascontiguousarray(np.stack(outs)).astype(np.float32)
